# revision 9
# baseline (speedup 1.0000x reference)
"""Trainium2 Bass kernel for GCN-biased sparse attention (nn_Attention_37589553775245).

Reference computation (per batch b of 8, one NeuronCore each):
    qkv = x @ w_qkv; q,k,v per head (H=8, DH=64)
    attn = softmax(q k^T / sqrt(DH)) + A_hat        (A_hat = D^-1/2 (ceil(adj)+I) D^-1/2)
    out = (attn @ v) @ w_out + b_out

Sharding: pure batch-parallel across the 8 cores (B=8). A_hat computed on host
(cheap) and replicated; weights replicated. No collectives.

v2 design notes (PE-bound roofline):
  - TimelineSim cost model: a matmul costs out_free_size cycles (f32r/bf16 at
    free>=256: 1 cyc/row @2.4GHz) regardless of contraction depth or output
    partition count. Total PE work here is 229376 rows ~= 95.6us; everything
    else (ACT exp stream 67.7us, DVE ~50us) fits underneath it, so the whole
    kernel is scheduled to keep the PE saturated.
  - All PE inputs are bf16 (except yT/yE/wout in f32r): same PE speed as
    f32r, half the DMA bytes and SBUF footprint. End-to-end numpy error
    ~3.8e-3 max-norm (gate 2e-2). fp8 was evaluated and rejected: every
    fp8 placement adds 1.3e-2..4.3e-2 max-norm error.
  - With bf16, everything is SBUF-resident from t=0: A_hat^T loads on the
    SWDGE ring in parallel with xT/wqkv on the sync ring, so the A_hat@V
    units are available as PE filler from ~8us (v1 stalled 10.8us on a
    deferred A_hat load).
  - scores are computed transposed per head: sT[j,i] = k_j . q_i with the
    softmax denominator riding the attn@v matmul as a ones column of vaug.
    vaug holds per-head 128-wide blocks [v_h | 1 | 0...] (even heads) and
    [1 | 0... | v_h@64:128] (odd heads), so each head's exp.V rows land
    directly at its yE partition base (0/64) and the denominator row (64/0)
    is recip'd + DRAM-bounce-broadcast to the matching partitions; no
    partition-shift copy is needed for either parity.
  - emission interleaves one stream of "filler" PE work (remaining q/k tiles,
    A_hat@V units, out-projection tiles) between the score/attn@v matmuls of
    each attention unit, so the PE never waits on the ACT exp stream.
  - out = Y @ w_out + b_out is emitted per 128-row tile as soon as its
    chunk's yT merges are done (chunk-0 projections weave into chunk-1
    attention).
"""

import os
import sys

import numpy as np

for _p in ("/opt/trn_rl_repo", "/root/.axon_site/_ro/trn_rl_repo"):
    if _p not in sys.path and os.path.isdir(_p):
        sys.path.insert(0, _p)

import ml_dtypes  # noqa: E402

import concourse.bass as bass  # noqa: E402
import concourse.mybir as mybir  # noqa: E402
import concourse.tile as tile  # noqa: E402
from concourse import bacc  # noqa: E402
from concourse.bass_utils import run_bass_kernel_spmd  # noqa: E402

B, N, DIM, H, DH = 8, 1024, 512, 8, 64
F = H * DH          # 512, inner dim
NT = N // 128       # 8 n-tiles (also j-tiles)
DT = DIM // 128     # 4 dim-tiles
FT = F // 128       # 4 f-tiles
NC2 = N // 512      # 2 i-chunks of 512
SCALE = DH ** -0.5

F32 = mybir.dt.float32
F32R = mybir.dt.float32r
BF16 = mybir.dt.bfloat16

_PROGRAM = None
_last_in_maps = None


def _build_program(reps=1):
    nc = bacc.Bacc("TRN2", target_bir_lowering=False, debug=False, num_devices=8)

    xT_d = nc.dram_tensor("xT", [DIM, N], BF16, kind="ExternalInput")
    wqkv_d = nc.dram_tensor("wqkv", [DIM, 3 * F], BF16, kind="ExternalInput")
    ahatT_d = nc.dram_tensor("ahatT", [N, N], BF16, kind="ExternalInput")
    wout_d = nc.dram_tensor("wout", [F, DIM], F32R, kind="ExternalInput")
    bout_d = nc.dram_tensor("bout", [1, DIM], F32, kind="ExternalInput")
    out_d = nc.dram_tensor("out", [N, DIM], F32, kind="ExternalOutput")

    with tile.TileContext(nc) as tc:
        with (
            tc.tile_pool(name="big", bufs=1) as big,
            tc.tile_pool(name="ps_mm", bufs=2, space="PSUM") as ps_mm,
            tc.tile_pool(name="ps_s", bufs=2, space="PSUM") as ps_s,
            tc.tile_pool(name="ps_o", bufs=2, space="PSUM") as ps_o,
        ):
          for _rep in range(reps):
            # ---- persistent SBUF tensors -------------------------------
            xT = big.tile([128, DT, N], BF16)
            wqkv = big.tile([128, DT, 3 * F], BF16)
            wout = big.tile([128, FT, DIM], F32R)
            qkT = big.tile([128, 2 * FT, N], BF16)     # [f, n] f=q(0:512),k(512:1024)
            v_sb = big.tile([128, NT, F], BF16)        # v[n, f]
            # per-head 128-col blocks: even h: [v|1|0..], odd h: [1|0..|v]
            vaug = big.tile([128, NT, FT, 2, 128], BF16)
            ahatT = big.tile([128, NT, N], BF16)
            yT = big.tile([128, FT, N], F32R)          # (A_hat V)^T then merged
            yE = big.tile([128, FT, N], F32R)          # normalized exp-attention
            bout_bc = big.tile([128, DIM], F32)

            exps = tc.alloc_tile_pool(name="exps", bufs=6)
            small = tc.alloc_tile_pool(name="small", bufs=4)
            outs = tc.alloc_tile_pool(name="outs", bufs=3)

            # ---- input DMAs (everything is SBUF-resident, bf16) --------
            # The cost model serializes all DMA transfers in one FIFO at
            # aggregate bandwidth, so emit every load on the sync ring in
            # the exact order emission consumes it: xT(dt01), wqkv q01,
            # xT(dt23), wqkv k01, v, q23, k23, then the late-needed
            # wout/bout/A_hat^T.
            def load_x_half(dh):
                nc.sync.dma_start(
                    out=xT[:, 2 * dh:2 * dh + 2, :],
                    in_=xT_d[dh * 256:(dh + 1) * 256, :].rearrange(
                        "(t p) n -> p t n", p=128),
                )

            def load_w_chunk(fc):
                nc.sync.dma_start(
                    out=wqkv[:, :, fc * 256:(fc + 1) * 256],
                    in_=wqkv_d[:, fc * 256:(fc + 1) * 256].rearrange(
                        "(t p) f -> p t f", p=128),
                )

            load_x_half(0)
            load_w_chunk(0)      # q tiles ft 0/1
            load_x_half(1)
            load_w_chunk(2)      # k tiles ft 4/5
            load_w_chunk(4)      # v
            load_w_chunk(5)
            load_w_chunk(1)      # q tiles ft 2/3
            load_w_chunk(3)      # k tiles ft 6/7
            nc.sync.dma_start(
                out=wout,
                in_=wout_d[:, :].rearrange("(t p) n -> p t n", p=128),
            )
            nc.sync.dma_start(out=bout_bc, in_=bout_d[0:1, :].to_broadcast((128, DIM)))
            nc.sync.dma_start(
                out=ahatT,
                in_=ahatT_d[:, :].rearrange("(t p) n -> p t n", p=128),
            )

            # vaug: ones into the denominator columns — one per 32-partition
            # block of the non-v half (even heads: cols 64/96 of the parity-0
            # block; odd: cols 0/32 of parity-1), so a single stream_shuffle
            # with mask [0]*32 broadcasts the recip'd denominator across the
            # 64 destination partitions. The rest of each block outside the v
            # columns is left uninitialized: those lhsT columns only feed
            # PSUM rows that are never read.
            nc.vector.memset(vaug[:, :, :, 0, 64:65], 1.0)
            nc.vector.memset(vaug[:, :, :, 0, 96:97], 1.0)
            nc.vector.memset(vaug[:, :, :, 1, 0:1], 1.0)
            nc.vector.memset(vaug[:, :, :, 1, 32:33], 1.0)

            # ---- builders ----------------------------------------------
            def emit_qk(ft):
                # qkT[:, ft, :] (one 128-row f-tile of q^T or k^T), 2 chunks
                for c in range(NC2):
                    ps = ps_mm.tile([128, 512], F32, tag="mm")
                    for dt_i in range(DT):
                        nc.tensor.matmul(
                            ps,
                            wqkv[:, dt_i, ft * 128:(ft + 1) * 128],
                            xT[:, dt_i, c * 512:(c + 1) * 512],
                            start=(dt_i == 0),
                            stop=(dt_i == DT - 1),
                        )
                        yield
                    nc.vector.tensor_copy(out=qkT[:, ft, c * 512:(c + 1) * 512],
                                          in_=ps)

            def emit_v():
                for nt in range(NT):
                    ps = ps_mm.tile([128, 512], F32, tag="mm")
                    for dt_i in range(DT):
                        nc.tensor.matmul(
                            ps,
                            xT[:, dt_i, nt * 128:(nt + 1) * 128],
                            wqkv[:, dt_i, 2 * F:3 * F],
                            start=(dt_i == 0),
                            stop=(dt_i == DT - 1),
                        )
                        yield
                    nc.vector.tensor_copy(out=v_sb[:, nt, :], in_=ps)
                    ps_r = ps.rearrange("p (a b d) -> p a b d", a=FT, b=2)
                    nc.vector.tensor_copy(out=vaug[:, nt, :, 0, 0:DH],
                                          in_=ps_r[:, :, 0, :])
                    nc.vector.tensor_copy(out=vaug[:, nt, :, 1, DH:128],
                                          in_=ps_r[:, :, 1, :])

            def ahat_unit(ft, c):
                # yT[:, ft, c-chunk] = (A_hat @ V)^T tile
                ps = ps_mm.tile([128, 512], F32, tag="mm")
                for jt in range(NT):
                    nc.tensor.matmul(
                        ps,
                        v_sb[:, jt, ft * 128:(ft + 1) * 128],
                        ahatT[:, jt, c * 512:(c + 1) * 512],
                        start=(jt == 0),
                        stop=(jt == NT - 1),
                    )
                    yield
                nc.vector.tensor_copy(out=yT[:, ft, c * 512:(c + 1) * 512], in_=ps)

            def out_proj(nt):
                ps = ps_mm.tile([128, 512], F32, tag="mm")
                for ft in range(FT):
                    nc.tensor.matmul(
                        ps,
                        yT[:, ft, nt * 128:(nt + 1) * 128],
                        wout[:, ft, :],
                        start=(ft == 0),
                        stop=(ft == FT - 1),
                    )
                    yield
                ot = outs.tile([128, DIM], F32, tag="ot")
                nc.vector.tensor_add(ot, ps, bout_bc)
                nc.sync.dma_start(out=out_d[nt * 128:(nt + 1) * 128, :], in_=ot)

            def merge(ft, c):
                # yT += yE on the finished chunk (DVE, all-SBUF 2x mode)
                sl = slice(c * 512, (c + 1) * 512)
                nc.vector.tensor_add(yT[:, ft, sl], yT[:, ft, sl], yE[:, ft, sl])

            class Fill:
                """One stream of filler PE work, pulled one matmul at a time."""

                def __init__(self, gens):
                    self.gens = list(gens)

                def pull(self, n):
                    while self.gens and n > 0:
                        try:
                            next(self.gens[0])
                            n -= 1
                        except StopIteration:
                            self.gens.pop(0)

                def drain(self):
                    for g in self.gens:
                        for _ in g:
                            pass
                    self.gens = []

            def attn_unit(h, c, fill):
                # one head, one 512-wide i-chunk; pulls PE filler work from
                # `fill` while ACT computes each exp batch.
                ht, par = h // 2, h % 2
                hb = par * 64
                dr = 64 if par == 0 else 0      # denominator row in ps_out
                ps_out = ps_o.tile([128, 512], F32, tag="po")
                n_jb = NT // 2
                ets = [None] * n_jb

                def scores(jb):
                    ps_sc = ps_s.tile([128, 2, 512], F32, tag="ps")
                    for e in range(2):
                        jt = jb * 2 + e
                        nc.tensor.matmul(
                            ps_sc[:, e, :],
                            qkT[hb:hb + 64, FT + ht, jt * 128:(jt + 1) * 128],
                            qkT[hb:hb + 64, ht, c * 512:(c + 1) * 512],
                        )
                    et = exps.tile([128, 2, 512], BF16, tag="exp")
                    nc.scalar.activation(out=et, in_=ps_sc,
                                         func=mybir.ActivationFunctionType.Exp,
                                         scale=float(SCALE))
                    ets[jb] = et

                def attnv(jb):
                    for e in range(2):
                        jt = jb * 2 + e
                        nc.tensor.matmul(
                            ps_out,
                            vaug[:, jt, ht, par, :],
                            ets[jb][:, e, :],
                            start=(jt == 0),
                            stop=(jt == NT - 1),
                        )

                # software-pipelined: scores(jb+1) and fillers run on the PE
                # while ACT exps batch jb; attnv(jb) follows.
                scores(0)
                fill.pull(2)
                for jb in range(1, n_jb):
                    scores(jb)
                    fill.pull(2)
                    attnv(jb - 1)
                attnv(n_jb - 1)

                # tail: recip the denominator window (rows dr/dr+32 hold the
                # denominator; the rest is junk, recip'd harmlessly), then
                # stream_shuffle-broadcast it across the 64 v partitions and
                # normalize into yE.
                rt = small.tile([128, 512], F32, tag="rt")
                nc.vector.reciprocal(out=rt[dr:dr + 64, :], in_=ps_out[dr:dr + 64, :])
                bc = small.tile([128, 512], F32, tag="bc")
                nc.vector.stream_shuffle(out=bc[hb:hb + 64, :], in_=rt[dr:dr + 64, :],
                                         mask=[0] * 32)
                nc.vector.tensor_mul(yE[hb:hb + 64, ht, c * 512:(c + 1) * 512],
                                     ps_out[hb:hb + 64, :], bc[hb:hb + 64, :])

            # ---- emission ----------------------------------------------
            def run(gen):
                for _ in gen:
                    pass

            run(emit_qk(0))   # q heads 0/1
            run(emit_qk(4))   # k heads 0/1
            run(emit_v())

            # chunk-0 attention; filler: remaining q/k tiles (qk(ft) must
            # complete before the units of heads 2ft/2ft+1 start), then the
            # first ahat units (ahatT has arrived by then)
            fill = Fill([
                emit_qk(1), emit_qk(5),    # q/k heads 2/3
                emit_qk(2), emit_qk(6),    # q/k heads 4/5
                emit_qk(3), emit_qk(7),    # q/k heads 6/7
                ahat_unit(0, 0), ahat_unit(1, 0),
            ])
            for h in range(H):
                attn_unit(h, 0, fill)
            fill.drain()
            merge(0, 0)   # ahat(0,0)/(1,0) emitted; tails of h0..h3 done
            merge(1, 0)

            # chunk-1 attention; filler: remaining ahat units, then chunk-0
            # out-projections (their merges complete during this chunk)
            fill = Fill([
                ahat_unit(2, 0), ahat_unit(3, 0),
                ahat_unit(0, 1), ahat_unit(1, 1),
                ahat_unit(2, 1), ahat_unit(3, 1),
                out_proj(0), out_proj(1), out_proj(2), out_proj(3),
            ])
            for h in range(H):
                attn_unit(h, 1, fill)
                if h == 1:
                    merge(2, 0)   # ahat(2,0) emitted during unit h=0
                elif h == 2:
                    merge(3, 0)
                elif h == 4:
                    merge(0, 1)   # ahat(0,1) emitted by unit h=3; tails h0/h1
                elif h == 6:
                    merge(1, 1)
            fill.drain()
            merge(2, 1)
            merge(3, 1)
            for nt in range(4, NT):
                run(out_proj(nt))

            outs.release()
            small.release()
            exps.release()

    nc.compile()
    return nc


def _get_program():
    global _PROGRAM
    if _PROGRAM is None:
        _PROGRAM = _build_program()
    return _PROGRAM


def kernel(x, adj, w_qkv, w_out, b_out):
    x = np.asarray(x, dtype=np.float32)
    adj = np.asarray(adj, dtype=np.float32)
    w_qkv = np.asarray(w_qkv, dtype=np.float32)
    w_out = np.ascontiguousarray(np.asarray(w_out, dtype=np.float32))
    b_out = np.asarray(b_out, dtype=np.float32).reshape(1, DIM)

    # host-side: normalized adjacency bias, replicated (one 1024^2 pass)
    A = np.ceil(adj) + np.eye(N, dtype=np.float32)
    dinv = A.sum(axis=1) ** -0.5
    A_hat = (A * dinv[:, None]) * dinv[None, :]
    ahatT = np.ascontiguousarray(A_hat.T).astype(ml_dtypes.bfloat16)

    wqkv_b = np.ascontiguousarray(w_qkv).astype(ml_dtypes.bfloat16)

    nc = _get_program()
    in_maps = []
    for b in range(B):
        in_maps.append({
            "xT": np.ascontiguousarray(x[b].T).astype(ml_dtypes.bfloat16),
            "wqkv": wqkv_b,
            "ahatT": ahatT,
            "wout": w_out,
            "bout": b_out,
        })
    global _last_in_maps
    _last_in_maps = in_maps
    res = run_bass_kernel_spmd(nc, in_maps, list(range(B)))
    out = np.stack([res.results[b]["out"] for b in range(B)], axis=0)
    return out.astype(np.float32)


if __name__ == "__main__":
    rng = np.random.default_rng(0)
    x = rng.standard_normal((B, N, DIM), dtype=np.float32)
    adj = (rng.random((N, N), dtype=np.float32) < 0.05).astype(np.float32) * 0.5
    w_qkv = rng.standard_normal((DIM, 3 * F), dtype=np.float32) * DIM ** -0.5
    w_out = rng.standard_normal((F, DIM), dtype=np.float32) * F ** -0.5
    b_out = np.zeros(DIM, dtype=np.float32)
    out = kernel(x=x, adj=adj, w_qkv=w_qkv, w_out=w_out, b_out=b_out)
    print("out", out.shape, out.dtype, np.abs(out).max())
